# revision 13
# baseline (speedup 1.0000x reference)
"""AttnBlock (GroupNorm + single-head self-attention + residual) on 8 TRN2 cores.

Strategy: data-parallel over batch (b=8) — one NeuronCore per batch element,
no collectives. Per core, everything is computed in [c, pos] layout so no
transposes are needed anywhere:

  - GroupNorm stats via free-dim DVE reductions + a tiny grouping matmul to
    sum across the 16 channels (partitions) of each group; per-channel
    scale/bias applied with one ScalarE pass.
  - q/k/v projections as matmuls with host-pre-transposed weights. q gets the
    attention scale (c^-0.5) folded into wq host-side; v's bias is folded into
    the final projection bias host-side (rows of softmax sum to 1).
  - scores are computed TRANSPOSED: S_T[k_pos, q_pos] = k^T q, with K=c on
    partitions -> exp directly on the PSUM->SBUF evacuation (ScalarE), no
    max-subtraction (scores ~ N(0,1), fp32-safe).
  - softmax denominators via a ones-vector matmul (partition-dim sum on PE),
    reciprocal on DVE, replicated across partitions with a K=1 matmul.
  - attn@v lands directly in [c, q_pos] layout: lhsT=v^T tile, rhs=E tile.
  - final projection + bias + residual, streamed back to HBM.

Big matmuls run in float32r (full-rate fp32, ~1.5e-4 rel err measured);
tiny stats matmuls in exact fp32.
"""
import sys

sys.path.insert(0, '/opt/trn_rl_repo')

import numpy as np

import concourse.bass as bass
import concourse.mybir as mybir
import concourse.tile as tile
from concourse import bacc, bass_utils

P = 128
F32 = mybir.dt.float32
F32R = mybir.dt.float32r
AF = mybir.ActivationFunctionType
ALU = mybir.AluOpType


def build_attn_kernel(c=512, n=4096, groups=32, eps=1e-6, pb=512, qb=256,
                      mm_dt=F32R, stop_after=None):
    """Build the per-core Bass program. Returns finalized nc.

    stop_after: None (full), 'p1' (GN stats only), 'p2' (through projections)
    — debug bisection: later phases are skipped and 'out' is filled from
    whatever is available.
    """
    cs = c // P                 # channel subtiles (4)
    nbp = n // pb               # projection-phase position blocks
    nqb = n // qb               # attention q blocks
    kt_n = n // P               # k-position tiles (32)
    gps = P // (c // groups)    # groups per channel-subtile (8)

    nc = bacc.Bacc("TRN2", target_bir_lowering=False, debug=False,
                   enable_asserts=False)

    x_d = nc.dram_tensor("x", (c, n), F32, kind="ExternalInput").ap()
    wq_d = nc.dram_tensor("wqt", (c, c), mm_dt, kind="ExternalInput").ap()
    wk_d = nc.dram_tensor("wkt", (c, c), mm_dt, kind="ExternalInput").ap()
    wv_d = nc.dram_tensor("wvt", (c, c), mm_dt, kind="ExternalInput").ap()
    wp_d = nc.dram_tensor("wpt", (c, c), mm_dt, kind="ExternalInput").ap()
    bqs_d = nc.dram_tensor("bqs", (P, cs), F32, kind="ExternalInput").ap()
    bks_d = nc.dram_tensor("bks", (P, cs), F32, kind="ExternalInput").ap()
    bps_d = nc.dram_tensor("bps", (P, cs), F32, kind="ExternalInput").ap()
    gws_d = nc.dram_tensor("gws", (P, cs), F32, kind="ExternalInput").ap()
    gbs_d = nc.dram_tensor("gbs", (P, cs), F32, kind="ExternalInput").ap()
    gm_d = nc.dram_tensor("gmat", (P, gps), F32, kind="ExternalInput").ap()
    gm2_d = nc.dram_tensor("gmat2", (gps, P), F32, kind="ExternalInput").ap()
    out_d = nc.dram_tensor("out", (c, n), F32, kind="ExternalOutput").ap()

    with tile.TileContext(nc) as tc:
        cpool = tc.alloc_tile_pool(name="const", bufs=1)
        kpool = tc.alloc_tile_pool(name="kfull", bufs=1)
        vpool = tc.alloc_tile_pool(name="vtfull", bufs=1)
        dpool = tc.alloc_tile_pool(name="dram", bufs=1, space="DRAM")

        q_dram = dpool.tile([P, cs, n], mm_dt)
        k_full = kpool.tile([P, cs, n], mm_dt)
        vt_full = vpool.tile([P, kt_n, c], mm_dt)

        bqs = cpool.tile([P, cs], F32)
        bks = cpool.tile([P, cs], F32)
        bps = cpool.tile([P, cs], F32)
        gws = cpool.tile([P, cs], F32)
        gbs = cpool.tile([P, cs], F32)
        gm = cpool.tile([P, gps], F32)
        gm2 = cpool.tile([gps, P], F32)
        for t, d in ((bqs, bqs_d), (bks, bks_d), (bps, bps_d), (gws, gws_d),
                     (gbs, gbs_d), (gm, gm_d), (gm2, gm2_d)):
            nc.sync.dma_start(t[:], d)

        ones_f = cpool.tile([P, 1], F32)
        nc.vector.memset(ones_f[:], 1.0)
        ones_r = cpool.tile([P, 1], mm_dt)
        nc.vector.tensor_copy(ones_r[:], ones_f[:])
        ones_row = cpool.tile([1, P], F32)
        nc.vector.memset(ones_row[:], 1.0)

        # ---------------- Phase 1: GroupNorm statistics ----------------
        s1 = cpool.tile([P, cs, nbp], F32)
        s2 = cpool.tile([P, cs, nbp], F32)
        with tc.tile_pool(name="p1", bufs=4) as p1, \
             tc.tile_pool(name="ps1", bufs=2, space="PSUM") as ps1:
            for ci in range(cs):
                for j in range(nbp):
                    xt = p1.tile([P, pb], F32, tag="x1")
                    nc.sync.dma_start(
                        xt[:], x_d[ci * P:(ci + 1) * P, j * pb:(j + 1) * pb])
                    nc.vector.reduce_sum(
                        out=s1[:, ci, j:j + 1], in_=xt[:],
                        axis=mybir.AxisListType.X)
                    sq = p1.tile([P, pb], F32, tag="sq")
                    nc.scalar.activation(sq[:], xt[:], AF.Square,
                                         accum_out=s2[:, ci, j:j + 1])
            # per-(channel, ci) totals
            st = cpool.tile([P, 2 * cs], F32)
            nc.vector.reduce_sum(out=st[:, 0:cs], in_=s1[:],
                                 axis=mybir.AxisListType.X)
            nc.vector.reduce_sum(out=st[:, cs:2 * cs], in_=s2[:],
                                 axis=mybir.AxisListType.X)
            # group sums across partitions: [gps, 2cs] = gm^T @ st
            psg = ps1.tile([gps, 2 * cs], F32)
            nc.tensor.matmul(psg[:], gm[:], st[:], start=True, stop=True)
            gsb = cpool.tile([gps, 2 * cs], F32)
            nc.vector.tensor_copy(gsb[:], psg[:])
            inv_cnt = 1.0 / (n * (c // groups))
            mean = cpool.tile([gps, cs], F32)
            e2 = cpool.tile([gps, cs], F32)
            nc.vector.tensor_scalar_mul(mean[:], gsb[:, 0:cs], inv_cnt)
            nc.vector.tensor_scalar_mul(e2[:], gsb[:, cs:2 * cs], inv_cnt)
            var = cpool.tile([gps, cs], F32)
            nc.vector.tensor_tensor(var[:], mean[:], mean[:], ALU.mult)
            nc.vector.tensor_tensor(var[:], e2[:], var[:], ALU.subtract)
            std = cpool.tile([gps, cs], F32)
            eps_t = cpool.tile([P, 1], F32)
            nc.vector.memset(eps_t[:], float(eps))
            nc.scalar.activation(std[:], var[:], AF.Sqrt, bias=eps_t[:gps, :])
            # st2 = [rstd | -mean*rstd]
            st2 = cpool.tile([gps, 2 * cs], F32)
            nc.vector.reciprocal(st2[:, 0:cs], std[:])
            nc.vector.tensor_tensor(st2[:, cs:2 * cs], mean[:], st2[:, 0:cs],
                                    ALU.mult)
            nc.vector.tensor_scalar_mul(st2[:, cs:2 * cs], st2[:, cs:2 * cs],
                                        -1.0)
            # broadcast to channels: [P, 2cs] = gm2^T @ st2
            psb = ps1.tile([P, 2 * cs], F32)
            nc.tensor.matmul(psb[:], gm2[:], st2[:], start=True, stop=True)
            bc = cpool.tile([P, 2 * cs], F32)
            nc.vector.tensor_copy(bc[:], psb[:])
            # per-channel scale a = gw*rstd, bias b = gb + gw*(-mean*rstd)
            a_ch = cpool.tile([P, cs], F32)
            b_ch = cpool.tile([P, cs], F32)
            nc.vector.tensor_tensor(a_ch[:], gws[:], bc[:, 0:cs], ALU.mult)
            nc.vector.tensor_tensor(b_ch[:], gws[:], bc[:, cs:2 * cs], ALU.mult)
            nc.vector.tensor_tensor(b_ch[:], b_ch[:], gbs[:], ALU.add)

        if stop_after == 'p1':
            dbg = cpool.tile([P, 2 * cs], F32)
            nc.vector.tensor_copy(dbg[:, 0:cs], a_ch[:])
            nc.vector.tensor_copy(dbg[:, cs:2 * cs], b_ch[:])
            nc.sync.dma_start(out_d[0:P, 0:2 * cs], dbg[:])

        # ---------------- Phase 2: GN apply + q/k/v projections ----------------
        if stop_after != 'p1':
            phase2(nc, tc, cs, c, n, pb, mm_dt, wq_d, wk_d, wv_d, x_d, q_dram,
                   k_full, vt_full, a_ch, b_ch, bqs, bks)

        if stop_after in ('p2', 'p2v'):
            with tc.tile_pool(name="dbg2", bufs=2) as dbg2:
                if stop_after == 'p2':
                    for ci in range(cs):
                        t = dbg2.tile([P, n], F32, tag="d")
                        nc.vector.tensor_copy(t[:], k_full[:, ci, :])
                        nc.sync.dma_start(out_d[ci * P:(ci + 1) * P, :], t[:])
                else:
                    for kt in range(n // P):
                        t = dbg2.tile([P, c], F32, tag="d")
                        nc.vector.tensor_copy(t[:], vt_full[:, kt, :])
                        nc.sync.dma_start(
                            out_d[:, kt * P:(kt + 1) * P].rearrange(
                                "c p -> p c"), t[:])

        # ---------------- Phase 3: attention + output projection ----------------
        if stop_after is None:
            phase3(nc, tc, cs, c, n, qb, mm_dt, wp_d, x_d, out_d, q_dram,
                   k_full, vt_full, bps, ones_r, ones_row)

        for p in (dpool, vpool, kpool, cpool):
            p.release()

    nc.finalize()
    return nc


def phase2(nc, tc, cs, c, n, pb, mm_dt, wq_d, wk_d, wv_d, x_d, q_dram,
           k_full, vt_full, a_ch, b_ch, bqs, bks):
    nbp = n // pb
    with tc.tile_pool(name="wqkv", bufs=1) as wpool, \
             tc.tile_pool(name="p2h", bufs=2) as p2h, \
             tc.tile_pool(name="p2x", bufs=4) as p2x, \
             tc.tile_pool(name="p2e", bufs=3) as p2e, \
             tc.tile_pool(name="ps2", bufs=4, space="PSUM") as ps2:
            wq = wpool.tile([P, cs, c], mm_dt)
            wk = wpool.tile([P, cs, c], mm_dt)
            wv = wpool.tile([P, cs, c], mm_dt)
            nc.sync.dma_start(wq[:], wq_d.rearrange("(ci p) o -> p ci o", p=P))
            nc.sync.dma_start(wk[:], wk_d.rearrange("(ci p) o -> p ci o", p=P))
            nc.sync.dma_start(wv[:], wv_d.rearrange("(ci p) o -> p ci o", p=P))

            for j in range(nbp):
                hs = []
                for ci in range(cs):
                    xt = p2x.tile([P, pb], F32, tag="x2")
                    nc.sync.dma_start(
                        xt[:], x_d[ci * P:(ci + 1) * P, j * pb:(j + 1) * pb])
                    ht = p2h.tile([P, pb], mm_dt, tag=f"h{ci}")
                    nc.scalar.activation(ht[:], xt[:], AF.Identity,
                                         bias=b_ch[:, ci:ci + 1],
                                         scale=a_ch[:, ci:ci + 1])
                    hs.append(ht)
                # q (spilled to DRAM) and k (resident)
                for w_t, bias_t, is_q in ((wq, bqs, True), (wk, bks, False)):
                    for co in range(cs):
                        psq = ps2.tile([P, pb], F32, tag="proj")
                        for ci in range(cs):
                            nc.tensor.matmul(
                                psq[:], w_t[:, ci, co * P:(co + 1) * P],
                                hs[ci][:], start=(ci == 0), stop=(ci == cs - 1))
                        if is_q:
                            qe = p2e.tile([P, pb], mm_dt, tag="qe")
                            nc.scalar.activation(qe[:], psq[:], AF.Identity,
                                                 bias=bias_t[:, co:co + 1])
                            nc.sync.dma_start(
                                q_dram[:, co, j * pb:(j + 1) * pb], qe[:])
                        else:
                            nc.scalar.activation(
                                k_full[:, co, j * pb:(j + 1) * pb], psq[:],
                                AF.Identity, bias=bias_t[:, co:co + 1])
                # v^T tiles (bias folded into bps host-side)
                for pt in range(pb // P):
                    psv = ps2.tile([P, c], F32, tag="proj")
                    for ci in range(cs):
                        nc.tensor.matmul(
                            psv[:], hs[ci][:, pt * P:(pt + 1) * P], wv[:, ci, :],
                            start=(ci == 0), stop=(ci == cs - 1))
                    nc.vector.tensor_copy(
                        vt_full[:, j * (pb // P) + pt, :], psv[:])


def phase3(nc, tc, cs, c, n, qb, mm_dt, wp_d, x_d, out_d, q_dram,
           k_full, vt_full, bps, ones_r, ones_row):
    nqb = n // qb
    kt_n = n // P
    with tc.tile_pool(name="wp", bufs=1) as wppool, \
             tc.tile_pool(name="epool", bufs=1) as epool, \
             tc.tile_pool(name="p3", bufs=2) as p3, \
             tc.tile_pool(name="p3s", bufs=2) as p3s, \
             tc.tile_pool(name="pss", bufs=2, space="PSUM") as pss, \
             tc.tile_pool(name="pso", bufs=2, space="PSUM") as pso, \
             tc.tile_pool(name="psm", bufs=1, space="PSUM") as psm:
            wp = wppool.tile([P, cs, c], mm_dt)
            nc.sync.dma_start(wp[:], wp_d.rearrange("(ci p) o -> p ci o", p=P))

            for q_i in range(nqb):
                qs = slice(q_i * qb, (q_i + 1) * qb)
                qt = p3.tile([P, cs, qb], mm_dt, tag="qblk")
                nc.sync.dma_start(qt[:], q_dram[:, :, qs])
                # scores^T -> exp
                es = []
                for kt in range(kt_n):
                    ps_s = pss.tile([P, qb], F32, tag="s")
                    for co in range(cs):
                        nc.tensor.matmul(
                            ps_s[:], k_full[:, co, kt * P:(kt + 1) * P],
                            qt[:, co, :], start=(co == 0), stop=(co == cs - 1))
                    e_t = epool.tile([P, qb], mm_dt, tag=f"e{kt}")
                    nc.scalar.activation(e_t[:], ps_s[:], AF.Exp)
                    es.append(e_t)
                # softmax denominators: ones^T @ E (partition sum), recip,
                # replicate across partitions via K=1 matmul
                ps_sum = psm.tile([1, qb], F32, tag="sum")
                for kt in range(kt_n):
                    nc.tensor.matmul(ps_sum[:], ones_r[:], es[kt][:],
                                     start=(kt == 0), stop=(kt == kt_n - 1))
                recip = p3s.tile([1, qb], F32, tag="recip")
                nc.vector.reciprocal(recip[:], ps_sum[:])
                ps_rep = psm.tile([P, qb], F32, tag="rep")
                nc.tensor.matmul(ps_rep[:], ones_row[:], recip[:],
                                 start=True, stop=True)
                rrep = p3s.tile([P, qb], F32, tag="rrep")
                nc.vector.tensor_copy(rrep[:], ps_rep[:])
                # attn @ v -> [c, q_pos], normalized on evacuation
                att = p3.tile([P, cs, qb], mm_dt, tag="att")
                for co in range(cs):
                    ps_o = pso.tile([P, qb], F32, tag="o")
                    for kt in range(kt_n):
                        nc.tensor.matmul(
                            ps_o[:], vt_full[:, kt, co * P:(co + 1) * P],
                            es[kt][:], start=(kt == 0), stop=(kt == kt_n - 1))
                    nc.vector.tensor_tensor(att[:, co, :], ps_o[:], rrep[:],
                                            ALU.mult)
                # output projection + bias + residual
                for co in range(cs):
                    ps_p = pso.tile([P, qb], F32, tag="p")
                    for ci in range(cs):
                        nc.tensor.matmul(
                            ps_p[:], wp[:, ci, co * P:(co + 1) * P],
                            att[:, ci, :], start=(ci == 0), stop=(ci == cs - 1))
                    xr = p3s.tile([P, qb], F32, tag="xr")
                    nc.sync.dma_start(xr[:], x_d[co * P:(co + 1) * P, qs])
                    t1 = p3s.tile([P, qb], F32, tag="t1")
                    nc.vector.tensor_tensor(t1[:], ps_p[:], xr[:], ALU.add)
                    ot = p3s.tile([P, qb], F32, tag="ot")
                    nc.scalar.activation(ot[:], t1[:], AF.Identity,
                                         bias=bps[:, co:co + 1])
                    nc.sync.dma_start(out_d[co * P:(co + 1) * P, qs], ot[:])


def _prep_host_inputs(x, gn_weight, gn_bias, wq, bq, wk, bk, wv, bv, wp, bp,
                      c=512):
    """Host-side weight prep shared by all cores."""
    cs = c // P
    scale = 1.0 / np.sqrt(c)
    gpp = None  # groups-per-subtile handled via gmat shapes

    def stripe(v):  # [c] -> [P, cs] with v[ci*128 + p] at [p, ci]
        return np.ascontiguousarray(
            v.reshape(cs, P).T.astype(np.float32))

    common = {
        "wqt": np.ascontiguousarray((wq.T * scale).astype(np.float32)),
        "wkt": np.ascontiguousarray(wk.T.astype(np.float32)),
        "wvt": np.ascontiguousarray(wv.T.astype(np.float32)),
        "wpt": np.ascontiguousarray(wp.T.astype(np.float32)),
        "bqs": stripe(bq * scale),
        "bks": stripe(bk),
        "bps": stripe(bp + wp.astype(np.float64) @ bv.astype(np.float64)),
        "gws": stripe(gn_weight),
        "gbs": stripe(gn_bias),
    }
    gsize = 16  # channels per group (512/32)
    gps = P // gsize
    gmat = np.zeros((P, gps), np.float32)
    gmat[np.arange(P), np.arange(P) // gsize] = 1.0
    common["gmat"] = gmat
    common["gmat2"] = np.ascontiguousarray(gmat.T)
    return common


_NC_CACHE = {}


def kernel(x, gn_weight, gn_bias, wq, bq, wk, bk, wv, bv, wp, bp):
    b, c, h, w = x.shape
    n = h * w
    key = (c, n)
    if key not in _NC_CACHE:
        _NC_CACHE[key] = build_attn_kernel(c=c, n=n)
    nc = _NC_CACHE[key]

    common = _prep_host_inputs(x, gn_weight, gn_bias, wq, bq, wk, bk, wv, bv,
                               wp, bp, c=c)
    xf = np.ascontiguousarray(np.asarray(x, np.float32).reshape(b, c, n))
    in_maps = [{**common, "x": xf[i]} for i in range(b)]
    res = bass_utils.run_bass_kernel_spmd(nc, in_maps, core_ids=list(range(b)))
    out = np.stack([r["out"] for r in res.results])
    return out.reshape(b, c, h, w).astype(np.float32)


# revision 15
# speedup vs baseline: 1.3109x; 1.3109x over previous
"""AttnBlock (GroupNorm + single-head self-attention + residual) on 8 TRN2 cores.

Strategy: data-parallel over batch (b=8) — one NeuronCore per batch element,
no collectives. Per core, everything is computed in [c, pos] layout so no
transposes are needed anywhere:

  - GroupNorm stats via free-dim DVE reductions + a tiny grouping matmul to
    sum across the 16 channels (partitions) of each group; per-channel
    scale/bias applied with one ScalarE pass.
  - q/k/v projections as matmuls with host-pre-transposed weights. q gets the
    attention scale (c^-0.5) folded into wq host-side; v's bias is folded into
    the final projection bias host-side (rows of softmax sum to 1).
  - scores are computed TRANSPOSED: S_T[k_pos, q_pos] = k^T q, with K=c on
    partitions -> exp directly on the PSUM->SBUF evacuation (ScalarE), no
    max-subtraction (scores ~ N(0,1), fp32-safe).
  - softmax denominators via a ones-vector matmul (partition-dim sum on PE),
    reciprocal on DVE, replicated across partitions with a K=1 matmul.
  - attn@v lands directly in [c, q_pos] layout: lhsT=v^T tile, rhs=E tile.
  - final projection + bias + residual, streamed back to HBM.

Big matmuls run in float32r (full-rate fp32, ~1.5e-4 rel err measured);
tiny stats matmuls in exact fp32.
"""
import sys

sys.path.insert(0, '/opt/trn_rl_repo')

import numpy as np

import concourse.bass as bass
import concourse.mybir as mybir
import concourse.tile as tile
from concourse import bacc, bass_utils

P = 128
F32 = mybir.dt.float32
F32R = mybir.dt.float32r
AF = mybir.ActivationFunctionType
ALU = mybir.AluOpType


def build_attn_kernel(c=512, n=4096, groups=32, eps=1e-6, pb=512, qb=256,
                      mm_dt=F32R, stop_after=None, repeat=1):
    """Build the per-core Bass program. Returns finalized nc.

    stop_after: None (full), 'p1' (GN stats only), 'p2' (through projections)
    — debug bisection: later phases are skipped and 'out' is filled from
    whatever is available.
    """
    cs = c // P                 # channel subtiles (4)
    nbp = n // pb               # projection-phase position blocks
    nqb = n // qb               # attention q blocks
    kt_n = n // P               # k-position tiles (32)
    gps = P // (c // groups)    # groups per channel-subtile (8)

    nc = bacc.Bacc("TRN2", target_bir_lowering=False, debug=False,
                   enable_asserts=False)

    x_d = nc.dram_tensor("x", (c, n), F32, kind="ExternalInput").ap()
    wq_d = nc.dram_tensor("wqt", (c, c), mm_dt, kind="ExternalInput").ap()
    wk_d = nc.dram_tensor("wkt", (c, c), mm_dt, kind="ExternalInput").ap()
    wv_d = nc.dram_tensor("wvt", (c, c), mm_dt, kind="ExternalInput").ap()
    wp_d = nc.dram_tensor("wpt", (c, c), mm_dt, kind="ExternalInput").ap()
    bqs_d = nc.dram_tensor("bqs", (P, cs), F32, kind="ExternalInput").ap()
    bks_d = nc.dram_tensor("bks", (P, cs), F32, kind="ExternalInput").ap()
    bps_d = nc.dram_tensor("bps", (P, cs), F32, kind="ExternalInput").ap()
    gws_d = nc.dram_tensor("gws", (P, cs), F32, kind="ExternalInput").ap()
    gbs_d = nc.dram_tensor("gbs", (P, cs), F32, kind="ExternalInput").ap()
    gm_d = nc.dram_tensor("gmat", (P, gps), F32, kind="ExternalInput").ap()
    gm2_d = nc.dram_tensor("gmat2", (gps, P), F32, kind="ExternalInput").ap()
    out_d = nc.dram_tensor("out", (c, n), F32, kind="ExternalOutput").ap()

    with tile.TileContext(nc) as tc:
        cpool = tc.alloc_tile_pool(name="const", bufs=1)
        kpool = tc.alloc_tile_pool(name="kfull", bufs=1)
        vpool = tc.alloc_tile_pool(name="vtfull", bufs=1)
        dpool = tc.alloc_tile_pool(name="dram", bufs=1, space="DRAM")

        q_dram = dpool.tile([P, cs, n], mm_dt)
        k_full = kpool.tile([P, cs, n], mm_dt)
        vt_full = vpool.tile([P, kt_n, c], mm_dt)

        bqs = cpool.tile([P, cs], F32)
        bks = cpool.tile([P, cs], F32)
        bps = cpool.tile([P, cs], F32)
        gws = cpool.tile([P, cs], F32)
        gbs = cpool.tile([P, cs], F32)
        gm = cpool.tile([P, gps], F32)
        gm2 = cpool.tile([gps, P], F32)
        for t, d in ((bqs, bqs_d), (bks, bks_d), (bps, bps_d), (gws, gws_d),
                     (gbs, gbs_d), (gm, gm_d), (gm2, gm2_d)):
            nc.sync.dma_start(t[:], d)

        ones_f = cpool.tile([P, 1], F32)
        nc.vector.memset(ones_f[:], 1.0)
        ones_r = cpool.tile([P, 1], mm_dt)
        nc.vector.tensor_copy(ones_r[:], ones_f[:])
        ones_row = cpool.tile([1, P], F32)
        nc.vector.memset(ones_row[:], 1.0)

        # ---------------- Phase 1: GroupNorm statistics ----------------
        s1 = cpool.tile([P, cs, nbp], F32)
        s2 = cpool.tile([P, cs, nbp], F32)
        with tc.tile_pool(name="p1", bufs=4) as p1, \
             tc.tile_pool(name="ps1", bufs=2, space="PSUM") as ps1:
            for ci in range(cs):
                for j in range(nbp):
                    xt = p1.tile([P, pb], F32, tag="x1")
                    nc.sync.dma_start(
                        xt[:], x_d[ci * P:(ci + 1) * P, j * pb:(j + 1) * pb])
                    nc.vector.reduce_sum(
                        out=s1[:, ci, j:j + 1], in_=xt[:],
                        axis=mybir.AxisListType.X)
                    sq = p1.tile([P, pb], F32, tag="sq")
                    nc.scalar.activation(sq[:], xt[:], AF.Square,
                                         accum_out=s2[:, ci, j:j + 1])
            # per-(channel, ci) totals
            st = cpool.tile([P, 2 * cs], F32)
            nc.vector.reduce_sum(out=st[:, 0:cs], in_=s1[:],
                                 axis=mybir.AxisListType.X)
            nc.vector.reduce_sum(out=st[:, cs:2 * cs], in_=s2[:],
                                 axis=mybir.AxisListType.X)
            # group sums across partitions: [gps, 2cs] = gm^T @ st
            psg = ps1.tile([gps, 2 * cs], F32)
            nc.tensor.matmul(psg[:], gm[:], st[:], start=True, stop=True)
            gsb = cpool.tile([gps, 2 * cs], F32)
            nc.vector.tensor_copy(gsb[:], psg[:])
            inv_cnt = 1.0 / (n * (c // groups))
            mean = cpool.tile([gps, cs], F32)
            e2 = cpool.tile([gps, cs], F32)
            nc.vector.tensor_scalar_mul(mean[:], gsb[:, 0:cs], inv_cnt)
            nc.vector.tensor_scalar_mul(e2[:], gsb[:, cs:2 * cs], inv_cnt)
            var = cpool.tile([gps, cs], F32)
            nc.vector.tensor_tensor(var[:], mean[:], mean[:], ALU.mult)
            nc.vector.tensor_tensor(var[:], e2[:], var[:], ALU.subtract)
            std = cpool.tile([gps, cs], F32)
            eps_t = cpool.tile([P, 1], F32)
            nc.vector.memset(eps_t[:], float(eps))
            nc.scalar.activation(std[:], var[:], AF.Sqrt, bias=eps_t[:gps, :])
            # st2 = [rstd | -mean*rstd]
            st2 = cpool.tile([gps, 2 * cs], F32)
            nc.vector.reciprocal(st2[:, 0:cs], std[:])
            nc.vector.tensor_tensor(st2[:, cs:2 * cs], mean[:], st2[:, 0:cs],
                                    ALU.mult)
            nc.vector.tensor_scalar_mul(st2[:, cs:2 * cs], st2[:, cs:2 * cs],
                                        -1.0)
            # broadcast to channels: [P, 2cs] = gm2^T @ st2
            psb = ps1.tile([P, 2 * cs], F32)
            nc.tensor.matmul(psb[:], gm2[:], st2[:], start=True, stop=True)
            bc = cpool.tile([P, 2 * cs], F32)
            nc.vector.tensor_copy(bc[:], psb[:])
            # per-channel scale a = gw*rstd, bias b = gb + gw*(-mean*rstd)
            a_ch = cpool.tile([P, cs], F32)
            b_ch = cpool.tile([P, cs], F32)
            nc.vector.tensor_tensor(a_ch[:], gws[:], bc[:, 0:cs], ALU.mult)
            nc.vector.tensor_tensor(b_ch[:], gws[:], bc[:, cs:2 * cs], ALU.mult)
            nc.vector.tensor_tensor(b_ch[:], b_ch[:], gbs[:], ALU.add)

        if stop_after == 'p1':
            dbg = cpool.tile([P, 2 * cs], F32)
            nc.vector.tensor_copy(dbg[:, 0:cs], a_ch[:])
            nc.vector.tensor_copy(dbg[:, cs:2 * cs], b_ch[:])
            nc.sync.dma_start(out_d[0:P, 0:2 * cs], dbg[:])

        # ---------------- Phase 2: GN apply + q/k/v projections ----------------
        if stop_after != 'p1':
            for _rep in range(repeat):
                phase2(nc, tc, cs, c, n, pb, mm_dt, wq_d, wk_d, wv_d, x_d,
                       q_dram, k_full, vt_full, a_ch, b_ch, bqs, bks)
                if stop_after is None and _rep < repeat - 1:
                    phase3(nc, tc, cs, c, n, qb, mm_dt, wp_d, x_d, out_d,
                           q_dram, k_full, vt_full, bps, ones_r, ones_row)

        if stop_after in ('p2', 'p2v'):
            with tc.tile_pool(name="dbg2", bufs=2) as dbg2:
                if stop_after == 'p2':
                    for ci in range(cs):
                        t = dbg2.tile([P, n], F32, tag="d")
                        nc.vector.tensor_copy(t[:], k_full[:, ci, :])
                        nc.sync.dma_start(out_d[ci * P:(ci + 1) * P, :], t[:])
                else:
                    for kt in range(n // P):
                        t = dbg2.tile([P, c], F32, tag="d")
                        nc.vector.tensor_copy(t[:], vt_full[:, kt, :])
                        nc.sync.dma_start(
                            out_d[:, kt * P:(kt + 1) * P].rearrange(
                                "c p -> p c"), t[:])

        # ---------------- Phase 3: attention + output projection ----------------
        if stop_after is None:
            phase3(nc, tc, cs, c, n, qb, mm_dt, wp_d, x_d, out_d, q_dram,
                   k_full, vt_full, bps, ones_r, ones_row)

        for p in (dpool, vpool, kpool, cpool):
            p.release()

    nc.finalize()
    return nc


def phase2(nc, tc, cs, c, n, pb, mm_dt, wq_d, wk_d, wv_d, x_d, q_dram,
           k_full, vt_full, a_ch, b_ch, bqs, bks):
    nbp = n // pb
    with tc.tile_pool(name="wqkv", bufs=1) as wpool, \
             tc.tile_pool(name="p2h", bufs=2) as p2h, \
             tc.tile_pool(name="p2x", bufs=4) as p2x, \
             tc.tile_pool(name="p2e", bufs=3) as p2e, \
             tc.tile_pool(name="ps2", bufs=4, space="PSUM") as ps2:
            wq = wpool.tile([P, cs, c], mm_dt)
            wk = wpool.tile([P, cs, c], mm_dt)
            wv = wpool.tile([P, cs, c], mm_dt)
            nc.sync.dma_start(wq[:], wq_d.rearrange("(ci p) o -> p ci o", p=P))
            nc.sync.dma_start(wk[:], wk_d.rearrange("(ci p) o -> p ci o", p=P))
            nc.sync.dma_start(wv[:], wv_d.rearrange("(ci p) o -> p ci o", p=P))

            for j in range(nbp):
                hs = []
                for ci in range(cs):
                    xt = p2x.tile([P, pb], F32, tag="x2")
                    nc.sync.dma_start(
                        xt[:], x_d[ci * P:(ci + 1) * P, j * pb:(j + 1) * pb])
                    ht = p2h.tile([P, pb], mm_dt, tag=f"h{ci}")
                    nc.scalar.activation(ht[:], xt[:], AF.Identity,
                                         bias=b_ch[:, ci:ci + 1],
                                         scale=a_ch[:, ci:ci + 1])
                    hs.append(ht)
                # q (spilled to DRAM) and k (resident)
                for w_t, bias_t, is_q in ((wq, bqs, True), (wk, bks, False)):
                    for co in range(cs):
                        psq = ps2.tile([P, pb], F32, tag="proj")
                        for ci in range(cs):
                            nc.tensor.matmul(
                                psq[:], w_t[:, ci, co * P:(co + 1) * P],
                                hs[ci][:], start=(ci == 0), stop=(ci == cs - 1))
                        if is_q:
                            qe = p2e.tile([P, pb], mm_dt, tag="qe")
                            nc.scalar.activation(qe[:], psq[:], AF.Identity,
                                                 bias=bias_t[:, co:co + 1])
                            nc.sync.dma_start(
                                q_dram[:, co, j * pb:(j + 1) * pb], qe[:])
                        else:
                            nc.scalar.activation(
                                k_full[:, co, j * pb:(j + 1) * pb], psq[:],
                                AF.Identity, bias=bias_t[:, co:co + 1])
                # v^T tiles (bias folded into bps host-side)
                for pt in range(pb // P):
                    psv = ps2.tile([P, c], F32, tag="proj")
                    for ci in range(cs):
                        nc.tensor.matmul(
                            psv[:], hs[ci][:, pt * P:(pt + 1) * P], wv[:, ci, :],
                            start=(ci == 0), stop=(ci == cs - 1))
                    nc.vector.tensor_copy(
                        vt_full[:, j * (pb // P) + pt, :], psv[:])


def phase3(nc, tc, cs, c, n, qb, mm_dt, wp_d, x_d, out_d, q_dram,
           k_full, vt_full, bps, ones_r, ones_row):
    nqb = n // qb
    kt_n = n // P
    with tc.tile_pool(name="wp", bufs=1) as wppool, \
             tc.tile_pool(name="epool", bufs=1) as epool, \
             tc.tile_pool(name="p3", bufs=2) as p3, \
             tc.tile_pool(name="p3s", bufs=2) as p3s, \
             tc.tile_pool(name="pss", bufs=2, space="PSUM") as pss, \
             tc.tile_pool(name="pso", bufs=2, space="PSUM") as pso, \
             tc.tile_pool(name="psm", bufs=1, space="PSUM") as psm:
            wp = wppool.tile([P, cs, c], mm_dt)
            nc.sync.dma_start(wp[:], wp_d.rearrange("(ci p) o -> p ci o", p=P))

            for q_i in range(nqb):
                qs = slice(q_i * qb, (q_i + 1) * qb)
                qt = p3.tile([P, cs, qb], mm_dt, tag="qblk")
                nc.sync.dma_start(qt[:], q_dram[:, :, qs])
                # scores^T -> exp
                es = []
                for kt in range(kt_n):
                    ps_s = pss.tile([P, qb], F32, tag="s")
                    for co in range(cs):
                        nc.tensor.matmul(
                            ps_s[:], k_full[:, co, kt * P:(kt + 1) * P],
                            qt[:, co, :], start=(co == 0), stop=(co == cs - 1))
                    e_t = epool.tile([P, qb], mm_dt, tag=f"e{kt}")
                    nc.scalar.activation(e_t[:], ps_s[:], AF.Exp)
                    es.append(e_t)
                # softmax denominators: ones^T @ E (partition sum), recip,
                # replicate across partitions via K=1 matmul
                ps_sum = psm.tile([1, qb], F32, tag="sum")
                for kt in range(kt_n):
                    nc.tensor.matmul(ps_sum[:], ones_r[:], es[kt][:],
                                     start=(kt == 0), stop=(kt == kt_n - 1))
                recip = p3s.tile([1, qb], F32, tag="recip")
                nc.vector.reciprocal(recip[:], ps_sum[:])
                ps_rep = psm.tile([P, qb], F32, tag="rep")
                nc.tensor.matmul(ps_rep[:], ones_row[:], recip[:],
                                 start=True, stop=True)
                rrep = p3s.tile([P, qb], F32, tag="rrep")
                nc.vector.tensor_copy(rrep[:], ps_rep[:])
                # attn @ v -> [c, q_pos], normalized on evacuation
                att = p3.tile([P, cs, qb], mm_dt, tag="att")
                for co in range(cs):
                    ps_o = pso.tile([P, qb], F32, tag="o")
                    for kt in range(kt_n):
                        nc.tensor.matmul(
                            ps_o[:], vt_full[:, kt, co * P:(co + 1) * P],
                            es[kt][:], start=(kt == 0), stop=(kt == kt_n - 1))
                    nc.vector.tensor_tensor(att[:, co, :], ps_o[:], rrep[:],
                                            ALU.mult)
                # output projection + bias + residual
                for co in range(cs):
                    ps_p = pso.tile([P, qb], F32, tag="p")
                    for ci in range(cs):
                        nc.tensor.matmul(
                            ps_p[:], wp[:, ci, co * P:(co + 1) * P],
                            att[:, ci, :], start=(ci == 0), stop=(ci == cs - 1))
                    xr = p3s.tile([P, qb], F32, tag="xr")
                    nc.sync.dma_start(xr[:], x_d[co * P:(co + 1) * P, qs])
                    t1 = p3s.tile([P, qb], F32, tag="t1")
                    nc.vector.tensor_tensor(t1[:], ps_p[:], xr[:], ALU.add)
                    ot = p3s.tile([P, qb], F32, tag="ot")
                    nc.scalar.activation(ot[:], t1[:], AF.Identity,
                                         bias=bps[:, co:co + 1])
                    nc.sync.dma_start(out_d[co * P:(co + 1) * P, qs], ot[:])


def _prep_host_inputs(x, gn_weight, gn_bias, wq, bq, wk, bk, wv, bv, wp, bp,
                      c=512):
    """Host-side weight prep shared by all cores."""
    cs = c // P
    scale = 1.0 / np.sqrt(c)
    gpp = None  # groups-per-subtile handled via gmat shapes

    def stripe(v):  # [c] -> [P, cs] with v[ci*128 + p] at [p, ci]
        return np.ascontiguousarray(
            v.reshape(cs, P).T.astype(np.float32))

    common = {
        "wqt": np.ascontiguousarray((wq.T * scale).astype(np.float32)),
        "wkt": np.ascontiguousarray(wk.T.astype(np.float32)),
        "wvt": np.ascontiguousarray(wv.T.astype(np.float32)),
        "wpt": np.ascontiguousarray(wp.T.astype(np.float32)),
        "bqs": stripe(bq * scale),
        "bks": stripe(bk),
        "bps": stripe(bp + wp.astype(np.float64) @ bv.astype(np.float64)),
        "gws": stripe(gn_weight),
        "gbs": stripe(gn_bias),
    }
    gsize = 16  # channels per group (512/32)
    gps = P // gsize
    gmat = np.zeros((P, gps), np.float32)
    gmat[np.arange(P), np.arange(P) // gsize] = 1.0
    common["gmat"] = gmat
    common["gmat2"] = np.ascontiguousarray(gmat.T)
    return common


_NC_CACHE = {}


def kernel(x, gn_weight, gn_bias, wq, bq, wk, bk, wv, bv, wp, bp):
    b, c, h, w = x.shape
    n = h * w
    key = (c, n)
    if key not in _NC_CACHE:
        _NC_CACHE[key] = build_attn_kernel(c=c, n=n)
    nc = _NC_CACHE[key]

    common = _prep_host_inputs(x, gn_weight, gn_bias, wq, bq, wk, bk, wv, bv,
                               wp, bp, c=c)
    xf = np.ascontiguousarray(np.asarray(x, np.float32).reshape(b, c, n))
    in_maps = [{**common, "x": xf[i]} for i in range(b)]
    res = bass_utils.run_bass_kernel_spmd(nc, in_maps, core_ids=list(range(b)))
    out = np.stack([r["out"] for r in res.results])
    return out.reshape(b, c, h, w).astype(np.float32)


# revision 22
# speedup vs baseline: 134.8737x; 102.8892x over previous
"""AttnBlock (GroupNorm + single-head self-attention + residual) on 8 TRN2 cores.

Strategy: data-parallel over batch (b=8) — one NeuronCore per batch element,
no collectives. Per core, everything is computed in [c, pos] layout so no
transposes are needed anywhere:

  - GroupNorm stats via free-dim DVE reductions + a tiny grouping matmul to
    sum across the 16 channels (partitions) of each group; per-channel
    scale/bias applied with one ScalarE pass.
  - q/k/v projections as matmuls with host-pre-transposed weights. q gets the
    attention scale (c^-0.5) folded into wq host-side; v's bias is folded into
    the final projection bias host-side (rows of softmax sum to 1).
  - scores are computed TRANSPOSED: S_T[k_pos, q_pos] = k^T q, with K=c on
    partitions -> exp directly on the PSUM->SBUF evacuation (ScalarE), no
    max-subtraction (scores ~ N(0,1), fp32-safe).
  - softmax denominators via a ones-vector matmul (partition-dim sum on PE),
    reciprocal on DVE, replicated across partitions with a K=1 matmul.
  - attn@v lands directly in [c, q_pos] layout: lhsT=v^T tile, rhs=E tile.
  - final projection + bias + residual, streamed back to HBM.

Big matmuls run in float32r (full-rate fp32, ~1.5e-4 rel err measured);
tiny stats matmuls in exact fp32.
"""
import sys

sys.path.insert(0, '/opt/trn_rl_repo')

import numpy as np

import concourse.mybir as mybir
import concourse.tile as tile
from concourse import bacc, bass_utils

P = 128
F32 = mybir.dt.float32
F32R = mybir.dt.float32r
AF = mybir.ActivationFunctionType
ALU = mybir.AluOpType


def build_attn_kernel(c=512, n=4096, groups=32, eps=1e-6, pb=512, qb=256,
                      mm_dt=F32R, stop_after=None, repeat=1):
    """Build the per-core Bass program. Returns finalized nc.

    stop_after: None (full), 'p1' (GN stats only), 'p2' (through projections)
    — debug bisection: later phases are skipped and 'out' is filled from
    whatever is available.
    """
    cs = c // P                 # channel subtiles (4)
    nbp = n // pb               # projection-phase position blocks
    nqb = n // qb               # attention q blocks
    kt_n = n // P               # k-position tiles (32)
    gps = P // (c // groups)    # groups per channel-subtile (8)

    nc = bacc.Bacc("TRN2", target_bir_lowering=False, debug=False,
                   enable_asserts=False)

    x_d = nc.dram_tensor("x", (c, n), F32, kind="ExternalInput").ap()
    wq_d = nc.dram_tensor("wqt", (c, c), mm_dt, kind="ExternalInput").ap()
    wk_d = nc.dram_tensor("wkt", (c, c), mm_dt, kind="ExternalInput").ap()
    wv_d = nc.dram_tensor("wvt", (c, c), mm_dt, kind="ExternalInput").ap()
    wp_d = nc.dram_tensor("wpt", (c, c), mm_dt, kind="ExternalInput").ap()
    bqs_d = nc.dram_tensor("bqs", (P, cs), F32, kind="ExternalInput").ap()
    bks_d = nc.dram_tensor("bks", (P, cs), F32, kind="ExternalInput").ap()
    bps_d = nc.dram_tensor("bps", (P, cs), F32, kind="ExternalInput").ap()
    gws_d = nc.dram_tensor("gws", (P, cs), F32, kind="ExternalInput").ap()
    gbs_d = nc.dram_tensor("gbs", (P, cs), F32, kind="ExternalInput").ap()
    gm_d = nc.dram_tensor("gmat", (P, gps), F32, kind="ExternalInput").ap()
    gm2_d = nc.dram_tensor("gmat2", (gps, P), F32, kind="ExternalInput").ap()
    out_d = nc.dram_tensor("out", (c, n), F32, kind="ExternalOutput").ap()

    with tile.TileContext(nc) as tc:
        cpool = tc.alloc_tile_pool(name="const", bufs=1)
        kpool = tc.alloc_tile_pool(name="kfull", bufs=1)
        vpool = tc.alloc_tile_pool(name="vtfull", bufs=1)
        dpool = tc.alloc_tile_pool(name="dram", bufs=1, space="DRAM")

        q_dram = dpool.tile([P, cs, n], mm_dt)
        k_full = kpool.tile([P, cs, n], mm_dt)
        vt_full = vpool.tile([P, kt_n, c], mm_dt)

        bqs = cpool.tile([P, cs], F32)
        bks = cpool.tile([P, cs], F32)
        bps = cpool.tile([P, cs], F32)
        gws = cpool.tile([P, cs], F32)
        gbs = cpool.tile([P, cs], F32)
        gm = cpool.tile([P, gps], F32)
        gm2 = cpool.tile([gps, P], F32)
        for t, d in ((bqs, bqs_d), (bks, bks_d), (bps, bps_d), (gws, gws_d),
                     (gbs, gbs_d), (gm, gm_d), (gm2, gm2_d)):
            nc.sync.dma_start(t[:], d)

        ones_mat = cpool.tile([P, P], F32)
        nc.vector.memset(ones_mat[:], 1.0)

        # ---------------- Phase 1: GroupNorm statistics ----------------
        s1 = cpool.tile([P, cs, nbp], F32)
        s2 = cpool.tile([P, cs, nbp], F32)
        with tc.tile_pool(name="p1", bufs=4) as p1, \
             tc.tile_pool(name="ps1", bufs=2, space="PSUM") as ps1:
            for ci in range(cs):
                for j in range(nbp):
                    xt = p1.tile([P, pb], F32, tag="x1")
                    nc.sync.dma_start(
                        xt[:], x_d[ci * P:(ci + 1) * P, j * pb:(j + 1) * pb])
                    nc.vector.reduce_sum(
                        out=s1[:, ci, j:j + 1], in_=xt[:],
                        axis=mybir.AxisListType.X)
                    sq = p1.tile([P, pb], F32, tag="sq")
                    nc.scalar.activation(sq[:], xt[:], AF.Square,
                                         accum_out=s2[:, ci, j:j + 1])
            # per-(channel, ci) totals
            st = cpool.tile([P, 2 * cs], F32)
            nc.vector.reduce_sum(out=st[:, 0:cs], in_=s1[:],
                                 axis=mybir.AxisListType.X)
            nc.vector.reduce_sum(out=st[:, cs:2 * cs], in_=s2[:],
                                 axis=mybir.AxisListType.X)
            # group sums across partitions: [gps, 2cs] = gm^T @ st
            psg = ps1.tile([gps, 2 * cs], F32)
            nc.tensor.matmul(psg[:], gm[:], st[:], start=True, stop=True)
            gsb = cpool.tile([gps, 2 * cs], F32)
            nc.vector.tensor_copy(gsb[:], psg[:])
            inv_cnt = 1.0 / (n * (c // groups))
            mean = cpool.tile([gps, cs], F32)
            e2 = cpool.tile([gps, cs], F32)
            nc.vector.tensor_scalar_mul(mean[:], gsb[:, 0:cs], inv_cnt)
            nc.vector.tensor_scalar_mul(e2[:], gsb[:, cs:2 * cs], inv_cnt)
            var = cpool.tile([gps, cs], F32)
            nc.vector.tensor_tensor(var[:], mean[:], mean[:], ALU.mult)
            nc.vector.tensor_tensor(var[:], e2[:], var[:], ALU.subtract)
            std = cpool.tile([gps, cs], F32)
            eps_t = cpool.tile([P, 1], F32)
            nc.vector.memset(eps_t[:], float(eps))
            nc.scalar.activation(std[:], var[:], AF.Sqrt, bias=eps_t[:gps, :])
            # st2 = [rstd | -mean*rstd]
            st2 = cpool.tile([gps, 2 * cs], F32)
            nc.vector.reciprocal(st2[:, 0:cs], std[:])
            nc.vector.tensor_tensor(st2[:, cs:2 * cs], mean[:], st2[:, 0:cs],
                                    ALU.mult)
            nc.vector.tensor_scalar_mul(st2[:, cs:2 * cs], st2[:, cs:2 * cs],
                                        -1.0)
            # broadcast to channels: [P, 2cs] = gm2^T @ st2
            psb = ps1.tile([P, 2 * cs], F32)
            nc.tensor.matmul(psb[:], gm2[:], st2[:], start=True, stop=True)
            bc = cpool.tile([P, 2 * cs], F32)
            nc.vector.tensor_copy(bc[:], psb[:])
            # per-channel scale a = gw*rstd, bias b = gb + gw*(-mean*rstd)
            a_ch = cpool.tile([P, cs], F32)
            b_ch = cpool.tile([P, cs], F32)
            nc.vector.tensor_tensor(a_ch[:], gws[:], bc[:, 0:cs], ALU.mult)
            nc.vector.tensor_tensor(b_ch[:], gws[:], bc[:, cs:2 * cs], ALU.mult)
            nc.vector.tensor_tensor(b_ch[:], b_ch[:], gbs[:], ALU.add)

        if stop_after == 'p1':
            dbg = cpool.tile([P, 2 * cs], F32)
            nc.vector.tensor_copy(dbg[:, 0:cs], a_ch[:])
            nc.vector.tensor_copy(dbg[:, cs:2 * cs], b_ch[:])
            nc.sync.dma_start(out_d[0:P, 0:2 * cs], dbg[:])

        # ---------------- Phase 2: GN apply + q/k/v projections ----------------
        if stop_after != 'p1':
            for _rep in range(repeat):
                phase2(nc, tc, cs, c, n, pb, mm_dt, wq_d, wk_d, wv_d, x_d,
                       q_dram, k_full, vt_full, a_ch, b_ch, bqs, bks)
                if stop_after is None and _rep < repeat - 1:
                    phase3(nc, tc, cs, c, n, qb, mm_dt, wp_d, x_d, out_d,
                           q_dram, k_full, vt_full, bps, ones_mat)

        if stop_after in ('p2', 'p2v'):
            with tc.tile_pool(name="dbg2", bufs=2) as dbg2:
                if stop_after == 'p2':
                    for ci in range(cs):
                        t = dbg2.tile([P, n], F32, tag="d")
                        nc.vector.tensor_copy(t[:], k_full[:, ci, :])
                        nc.sync.dma_start(out_d[ci * P:(ci + 1) * P, :], t[:])
                else:
                    for kt in range(n // P):
                        t = dbg2.tile([P, c], F32, tag="d")
                        nc.vector.tensor_copy(t[:], vt_full[:, kt, :])
                        nc.sync.dma_start(
                            out_d[:, kt * P:(kt + 1) * P].rearrange(
                                "c p -> p c"), t[:])

        # ---------------- Phase 3: attention + output projection ----------------
        if stop_after is None:
            phase3(nc, tc, cs, c, n, qb, mm_dt, wp_d, x_d, out_d, q_dram,
                   k_full, vt_full, bps, ones_mat)

        for p in (dpool, vpool, kpool, cpool):
            p.release()

    nc.finalize()
    return nc


def phase2(nc, tc, cs, c, n, pb, mm_dt, wq_d, wk_d, wv_d, x_d, q_dram,
           k_full, vt_full, a_ch, b_ch, bqs, bks):
    nbp = n // pb
    with tc.tile_pool(name="wqkv", bufs=1) as wpool, \
             tc.tile_pool(name="p2h", bufs=2) as p2h, \
             tc.tile_pool(name="p2x", bufs=4) as p2x, \
             tc.tile_pool(name="p2e", bufs=3) as p2e, \
             tc.tile_pool(name="ps2", bufs=4, space="PSUM") as ps2:
            wq = wpool.tile([P, cs, c], mm_dt)
            wk = wpool.tile([P, cs, c], mm_dt)
            wv = wpool.tile([P, cs, c], mm_dt)
            nc.sync.dma_start(wq[:], wq_d.rearrange("(ci p) o -> p ci o", p=P))
            nc.sync.dma_start(wk[:], wk_d.rearrange("(ci p) o -> p ci o", p=P))
            nc.sync.dma_start(wv[:], wv_d.rearrange("(ci p) o -> p ci o", p=P))

            for j in range(nbp):
                hs = []
                for ci in range(cs):
                    xt = p2x.tile([P, pb], F32, tag="x2")
                    nc.sync.dma_start(
                        xt[:], x_d[ci * P:(ci + 1) * P, j * pb:(j + 1) * pb])
                    ht = p2h.tile([P, pb], mm_dt, tag=f"h{ci}")
                    nc.scalar.activation(ht[:], xt[:], AF.Identity,
                                         bias=b_ch[:, ci:ci + 1],
                                         scale=a_ch[:, ci:ci + 1])
                    hs.append(ht)
                # q (spilled to DRAM) and k (resident)
                for w_t, bias_t, is_q in ((wq, bqs, True), (wk, bks, False)):
                    for co in range(cs):
                        psq = ps2.tile([P, pb], F32, tag="proj")
                        for ci in range(cs):
                            nc.tensor.matmul(
                                psq[:], w_t[:, ci, co * P:(co + 1) * P],
                                hs[ci][:], start=(ci == 0), stop=(ci == cs - 1))
                        if is_q:
                            qe = p2e.tile([P, pb], mm_dt, tag="qe")
                            nc.scalar.activation(qe[:], psq[:], AF.Identity,
                                                 bias=bias_t[:, co:co + 1])
                            nc.sync.dma_start(
                                q_dram[:, co, j * pb:(j + 1) * pb], qe[:])
                        else:
                            nc.scalar.activation(
                                k_full[:, co, j * pb:(j + 1) * pb], psq[:],
                                AF.Identity, bias=bias_t[:, co:co + 1])
                # v^T tiles (bias folded into bps host-side)
                for pt in range(pb // P):
                    psv = ps2.tile([P, c], F32, tag="proj")
                    for ci in range(cs):
                        nc.tensor.matmul(
                            psv[:], hs[ci][:, pt * P:(pt + 1) * P], wv[:, ci, :],
                            start=(ci == 0), stop=(ci == cs - 1))
                    nc.vector.tensor_copy(
                        vt_full[:, j * (pb // P) + pt, :], psv[:])


def phase3(nc, tc, cs, c, n, qb, mm_dt, wp_d, x_d, out_d, q_dram,
           k_full, vt_full, bps, ones_mat):
    nqb = n // qb
    kt_n = n // P
    with tc.tile_pool(name="wp", bufs=1) as wppool, \
             tc.tile_pool(name="epool", bufs=1) as epool, \
             tc.tile_pool(name="p3", bufs=2) as p3, \
             tc.tile_pool(name="p3s", bufs=2) as p3s, \
             tc.tile_pool(name="pss", bufs=2, space="PSUM") as pss, \
             tc.tile_pool(name="pso", bufs=2, space="PSUM") as pso, \
             tc.tile_pool(name="psm", bufs=1, space="PSUM") as psm:
            wp = wppool.tile([P, cs, c], mm_dt)
            nc.sync.dma_start(wp[:], wp_d.rearrange("(ci p) o -> p ci o", p=P))

            for q_i in range(nqb):
                qs = slice(q_i * qb, (q_i + 1) * qb)
                qt = p3.tile([P, cs, qb], mm_dt, tag="qblk")
                nc.sync.dma_start(qt[:], q_dram[:, :, qs])
                # scores^T -> exp
                es = []
                for kt in range(kt_n):
                    ps_s = pss.tile([P, qb], F32, tag="s")
                    for co in range(cs):
                        nc.tensor.matmul(
                            ps_s[:], k_full[:, co, kt * P:(kt + 1) * P],
                            qt[:, co, :], start=(co == 0), stop=(co == cs - 1))
                    e_t = epool.tile([P, qb], mm_dt, tag=f"e{kt}")
                    nc.scalar.activation(e_t[:], ps_s[:], AF.Exp)
                    es.append(e_t)
                # softmax denominators: accumulate E on DVE (overlaps with
                # exp), then one fp32 ones-matmul replicates the partition
                # sums across all 128 rows; reciprocal reads PSUM directly.
                tsum = p3s.tile([P, qb], F32, tag="tsum")
                nc.vector.tensor_tensor(tsum[:], es[0][:], es[1][:], ALU.add)
                for kt in range(2, kt_n):
                    nc.vector.tensor_tensor(tsum[:], tsum[:], es[kt][:],
                                            ALU.add)
                rrep = p3s.tile([P, qb], F32, tag="rrep")
                # attn @ v -> [c, q_pos], normalized on evacuation
                att = p3.tile([P, cs, qb], mm_dt, tag="att")
                for co in range(cs):
                    ps_o = pso.tile([P, qb], F32, tag="o")
                    for kt in range(kt_n):
                        nc.tensor.matmul(
                            ps_o[:], vt_full[:, kt, co * P:(co + 1) * P],
                            es[kt][:], start=(kt == 0), stop=(kt == kt_n - 1))
                    if co == 0:
                        ps_sum = psm.tile([P, qb], F32, tag="sum")
                        nc.tensor.matmul(ps_sum[:], ones_mat[:], tsum[:],
                                         start=True, stop=True)
                        nc.vector.reciprocal(rrep[:], ps_sum[:])
                    nc.vector.tensor_tensor(att[:, co, :], ps_o[:], rrep[:],
                                            ALU.mult)
                # output projection + bias + residual
                for co in range(cs):
                    ps_p = pso.tile([P, qb], F32, tag="p")
                    for ci in range(cs):
                        nc.tensor.matmul(
                            ps_p[:], wp[:, ci, co * P:(co + 1) * P],
                            att[:, ci, :], start=(ci == 0), stop=(ci == cs - 1))
                    xr = p3s.tile([P, qb], F32, tag="xr")
                    nc.sync.dma_start(xr[:], x_d[co * P:(co + 1) * P, qs])
                    t1 = p3s.tile([P, qb], F32, tag="t1")
                    nc.vector.tensor_tensor(t1[:], ps_p[:], xr[:], ALU.add)
                    ot = p3s.tile([P, qb], F32, tag="ot")
                    nc.scalar.activation(ot[:], t1[:], AF.Identity,
                                         bias=bps[:, co:co + 1])
                    nc.sync.dma_start(out_d[co * P:(co + 1) * P, qs], ot[:])


def _prep_host_inputs(x, gn_weight, gn_bias, wq, bq, wk, bk, wv, bv, wp, bp,
                      c=512):
    """Host-side weight prep shared by all cores."""
    cs = c // P
    scale = 1.0 / np.sqrt(c)

    def stripe(v):  # [c] -> [P, cs] with v[ci*128 + p] at [p, ci]
        return np.ascontiguousarray(
            v.reshape(cs, P).T.astype(np.float32))

    common = {
        "wqt": np.ascontiguousarray((wq.T * scale).astype(np.float32)),
        "wkt": np.ascontiguousarray(wk.T.astype(np.float32)),
        "wvt": np.ascontiguousarray(wv.T.astype(np.float32)),
        "wpt": np.ascontiguousarray(wp.T.astype(np.float32)),
        "bqs": stripe(bq * scale),
        "bks": stripe(bk),
        "bps": stripe(bp + wp.astype(np.float64) @ bv.astype(np.float64)),
        "gws": stripe(gn_weight),
        "gbs": stripe(gn_bias),
    }
    gsize = 16  # channels per group (512/32)
    gps = P // gsize
    gmat = np.zeros((P, gps), np.float32)
    gmat[np.arange(P), np.arange(P) // gsize] = 1.0
    common["gmat"] = gmat
    common["gmat2"] = np.ascontiguousarray(gmat.T)
    return common


_NC_CACHE = {}


def kernel(x, gn_weight, gn_bias, wq, bq, wk, bk, wv, bv, wp, bp):
    b, c, h, w = x.shape
    n = h * w
    key = (c, n)
    if key not in _NC_CACHE:
        _NC_CACHE[key] = build_attn_kernel(c=c, n=n)
    nc = _NC_CACHE[key]

    common = _prep_host_inputs(x, gn_weight, gn_bias, wq, bq, wk, bk, wv, bv,
                               wp, bp, c=c)
    xf = np.ascontiguousarray(np.asarray(x, np.float32).reshape(b, c, n))
    in_maps = [{**common, "x": xf[i]} for i in range(b)]
    res = bass_utils.run_bass_kernel_spmd(nc, in_maps, core_ids=list(range(b)))
    out = np.stack([r["out"] for r in res.results])
    return out.reshape(b, c, h, w).astype(np.float32)
